# revision 2
# baseline (speedup 1.0000x reference)
"""BlockTucker fusion kernel for 8 Trainium2 NeuronCores (v2).

Reference computation (per batch row b):
    h0 = x0 @ W0 + b0; h1 = x1 @ W1 + b1              # [B, 1600]
    per chunk c (20 chunks of 80):
        z[c,o] = sum_{s,t} h0c[s] Wb[c,o,s,t] h1c[t] + bb[c,o]
        z = signsqrt(z); z /= max(||z||_2, 1e-12)
    out = concat(z) @ Wout + bout                      # [B, 3000]

Strategy: pure data parallel over batch (1024 rows/core), bf16 compute.
The bilinear form is an outer-product matmul: per chunk,
P^T[(s,t), b] = h0[s,b]*h1[t,b] is built feature-major by DMA-replicating
rows of H^T across partitions (8 s-rows x 16 reps / 16 t-rows x 8 reps per
128-partition tile) and elementwise multiplies SPLIT between DVE and Pool
so production keeps pace with the tensor engine; then z^T[o,b] =
sum_{st} WbT[(s,t),o] P^T[(s,t),b] accumulates over 50 k-tiles in PSUM.

v2 changes vs v1 (997976 ns baseline):
  - per-chunk post-processing emitted one chunk behind the bilinear
    (software pipelining -> no in-order engine-queue stalls)
  - P-product planes split DVE:Pool ~3.4:1.6 (DVE alone was slower than PE)
  - DMA triggers spread across queues (h0rep on ACT, weights on SP,
    x-loads on Pool) and replication DMAs merged per chunk
  - all weights repacked host-side for contiguous per-partition
    descriptors (4-8KB); x passed bf16 from host (half the bytes)
  - per-chunk L2 norm (no 4-chunk batching), normalize applied in-place
    on the packed zbig tile
"""

import sys

sys.path.insert(0, "/opt/trn_rl_repo")

from contextlib import ExitStack

import numpy as np
import ml_dtypes

import concourse.bass as bass
import concourse.mybir as mybir
import concourse.tile as tile
from concourse import bacc
from concourse.bass_utils import run_bass_kernel_spmd

BF16 = mybir.dt.bfloat16
F32 = mybir.dt.float32
AF = mybir.ActivationFunctionType

B = 8192
D_IN = 2048
MM = 1600
CHUNKS = 20
CS = 80
D_OUT = 3000
N_CORES = 8
BL = B // N_CORES  # 1024 batch rows per core

K_IN = D_IN // 128  # 16 k-tiles for projections
MT_H = 13  # m-tiles for H (1600 -> 12x128 + 64)
MM_PAD = MT_H * 128  # 1664
KT_BIL = 50  # k-tiles per chunk for bilinear (6400/128)
MT_O = 24  # m-tiles for out (3000 -> 23x128 + 56)
D_OUT_PAD = MT_O * 128  # 3072
NH = BL // 512  # 2 free-dim halves of 512

# DVE planes per (i, half) P-tile; Pool builds the rest (5 - nd).
# DVE rate 0.521 ns/elem vs Pool 0.833: 3.4/1.6 split keeps both under
# the PE's 2.13 us per i-tile.
ND_PAT = [4, 3, 3, 4, 3, 4, 3, 3, 4, 3]


def _h_row_segments(mm0, nrows):
    """Split H rows [mm0, mm0+nrows) at 128-partition boundaries.

    Yields (row_off_in_block, p0, kt, n) for each contiguous segment.
    """
    segs = []
    a = mm0
    while a < mm0 + nrows:
        p0 = a % 128
        kt = a // 128
        n = min(128 - p0, mm0 + nrows - a)
        segs.append((a - mm0, p0, kt, n))
        a += n
    return segs


def build_program():
    nc = bacc.Bacc("TRN2", target_bir_lowering=False, debug=False)

    # DRAM parameters (per-core shards / replicated weights)
    x0T = nc.dram_tensor("x0T", [D_IN, BL], BF16, kind="ExternalInput").ap()
    x1T = nc.dram_tensor("x1T", [D_IN, BL], BF16, kind="ExternalInput").ap()
    w0p = nc.dram_tensor("w0p", [MT_H, 128, K_IN, 128], BF16, kind="ExternalInput").ap()
    w1p = nc.dram_tensor("w1p", [MT_H, 128, K_IN, 128], BF16, kind="ExternalInput").ap()
    wbp = nc.dram_tensor("wbp", [CHUNKS, 128, KT_BIL, CS], BF16, kind="ExternalInput").ap()
    woutp = nc.dram_tensor("woutp", [MT_O, 128, MT_H, 128], BF16, kind="ExternalInput").ap()
    b0c = nc.dram_tensor("b0c", [128, MT_H], F32, kind="ExternalInput").ap()
    b1c = nc.dram_tensor("b1c", [128, MT_H], F32, kind="ExternalInput").ap()
    bbT = nc.dram_tensor("bbT", [CS, CHUNKS], F32, kind="ExternalInput").ap()
    boutc = nc.dram_tensor("boutc", [128, MT_O], F32, kind="ExternalInput").ap()
    outT = nc.dram_tensor("outT", [D_OUT, BL], F32, kind="ExternalOutput").ap()
    rn_dram = nc.dram_tensor("rn_dram", [CHUNKS, BL], BF16).ap()
    h1_dram = nc.dram_tensor("h1_dram", [128, MT_H, BL], BF16).ap()

    with tile.TileContext(nc) as tc:
        _emit(
            tc, nc, x0T, x1T, w0p, w1p, wbp, woutp, b0c, b1c, bbT, boutc,
            outT, rn_dram, h1_dram,
        )
    nc.compile()
    return nc


def _emit(
    tc, nc, x0T, x1T, w0p, w1p, wbp, woutp, b0c, b1c, bbT, boutc, outT,
    rn_dram, h1_dram,
):
    ctx = ExitStack()
    with ctx:
        singles = ctx.enter_context(tc.tile_pool(name="singles", bufs=1))
        hpool = ctx.enter_context(tc.tile_pool(name="hpool", bufs=1))
        mm_psum = ctx.enter_context(tc.tile_pool(name="mm_psum", bufs=2, space="PSUM"))
        zpsum_pool = ctx.enter_context(tc.tile_pool(name="zpsum", bufs=2, space="PSUM"))
        # PSUM budget: mm_psum 2x1 + zps 2x2 + nps 1x2 = 8 banks exactly
        nsq_psum = ctx.enter_context(tc.tile_pool(name="nsq_psum", bufs=1, space="PSUM"))

        # constants / biases
        b0s = singles.tile([128, MT_H], F32)
        nc.sync.dma_start(out=b0s, in_=b0c)
        b1s = singles.tile([128, MT_H], F32)
        nc.sync.dma_start(out=b1s, in_=b1c)
        bbs = singles.tile([CS, CHUNKS], F32)
        nc.sync.dma_start(out=bbs, in_=bbT)
        bouts = singles.tile([128, MT_O], F32)
        nc.sync.dma_start(out=bouts, in_=boutc)
        ones80 = singles.tile([CS, 1], BF16)
        nc.vector.memset(ones80, 1.0)
        eps_sq = singles.tile([1, 1], F32)
        nc.vector.memset(eps_sq, 1e-24)

        # Z (normalized, repacked) for the final matmul: rows = c*80+o,
        # padded. Split at k-tile 8 so the phase-4 head start (kt 0..7,
        # final once chunk 12 lands) doesn't dep-chain on late chunks.
        KT_SPLIT = 8
        zbigA = singles.tile([128, KT_SPLIT, BL], BF16)
        zbigB = singles.tile([128, MT_H - KT_SPLIT, BL], BF16)
        nc.vector.memset(zbigB[64:128, MT_H - 1 - KT_SPLIT, :], 0.0)

        def zbig_slice(kt, cols=slice(None)):
            if kt < KT_SPLIT:
                return zbigA[:, kt, cols]
            return zbigB[:, kt - KT_SPLIT, cols]

        h0s = hpool.tile([128, MT_H, BL], BF16, tag="h0")

        # Pre-opened pools for the phase-1 -> phase-2 pipeline head: chunk
        # 0's replication tiles and the pt pool live outside the phase-1
        # pool scope, so their DMAs/multiplies run during the projections
        # instead of waiting for the phase-1 pool-release barrier.
        head_pool = ctx.enter_context(tc.tile_pool(name="head", bufs=1))
        p_pool = ctx.enter_context(tc.tile_pool(name="ppool", bufs=3))

        # ---- Phase 1: projections. h0 -> resident SBUF tile (feature-major
        # bf16); h1 -> per-m-tile evac straight to its DRAM mirror (read back
        # by the interleaved-replication DMAs). x loaded in 4-ktile groups
        # spread over queues; group 0 double-buffered so the second input's
        # head loads during the first input's tail. ----
        with tc.tile_pool(name="xg0", bufs=2) as xg0_pool, tc.tile_pool(
            name="xgr", bufs=1
        ) as xgr_pool, tc.tile_pool(name="wproj", bufs=2) as wproj, tc.tile_pool(
            name="h1e", bufs=2
        ) as h1e_pool:
            for inp_idx, (xT, wp, bias_s) in enumerate(
                ((x0T, w0p, b0s), (x1T, w1p, b1s))
            ):
                xr = xT.rearrange("(kt p) b -> p kt b", p=128)
                xg = []
                for g in range(4):
                    pool = xg0_pool if g == 0 else xgr_pool
                    t = pool.tile([128, 4, BL], BF16, tag=f"xs{g}")
                    q = (nc.gpsimd, nc.scalar, nc.gpsimd, nc.scalar)[g]
                    if g == 0 and inp_idx == 0:
                        # split for a faster first matmul
                        q.dma_start(out=t[:, 0:1, :], in_=xr[:, 0:1, :])
                        q.dma_start(out=t[:, 1:4, :], in_=xr[:, 1:4, :])
                    else:
                        q.dma_start(out=t, in_=xr[:, 4 * g : 4 * g + 4, :])
                    xg.append(t)
                for mt in range(MT_H):
                    wt = wproj.tile([128, K_IN, 128], BF16, tag="wt")
                    if inp_idx == 0 and mt == 0:
                        nc.sync.dma_start(out=wt[:, :2, :], in_=wp[mt][:, :2, :])
                        nc.sync.dma_start(out=wt[:, 2:, :], in_=wp[mt][:, 2:, :])
                    else:
                        nc.sync.dma_start(out=wt, in_=wp[mt])
                    h1ev = None
                    if inp_idx == 1:
                        h1ev = h1e_pool.tile([128, BL], BF16, tag="h1ev")
                    for h in range(NH):
                        ps = mm_psum.tile([128, 512], F32, tag="mmps")
                        for kt in range(K_IN):
                            nc.tensor.matmul(
                                out=ps,
                                lhsT=wt[:, kt, :],
                                rhs=xg[kt // 4][:, kt % 4, h * 512 : (h + 1) * 512],
                                start=(kt == 0),
                                stop=(kt == K_IN - 1),
                            )
                        dst = (
                            h0s[:, mt, h * 512 : (h + 1) * 512]
                            if inp_idx == 0
                            else h1ev[:, h * 512 : (h + 1) * 512]
                        )
                        nc.scalar.activation(
                            out=dst,
                            in_=ps,
                            func=AF.Identity,
                            bias=bias_s[:, mt : mt + 1],
                            scale=1.0,
                        )
                    if inp_idx == 1:
                        nc.sync.dma_start(out=h1_dram[:, mt, :], in_=h1ev)

        # ---- Phases 2+3: bilinear per chunk, post-processing emitted one
        # chunk behind (software pipelining). ----
        with ExitStack() as p23:
            dup_pool = p23.enter_context(tc.tile_pool(name="dup", bufs=2))
            rep0_pool = p23.enter_context(tc.tile_pool(name="rep0", bufs=2))
            rep1_pool = p23.enter_context(tc.tile_pool(name="rep1", bufs=1))
            wb_pool = p23.enter_context(tc.tile_pool(name="wbpool", bufs=2))
            post_pool = p23.enter_context(tc.tile_pool(name="post", bufs=2))
            zs_pool = p23.enter_context(tc.tile_pool(name="zs", bufs=2))
            nrm_pool = p23.enter_context(tc.tile_pool(name="nrm", bufs=1))
            rnb_pool = p23.enter_context(tc.tile_pool(name="rnb", bufs=1))

            def emit_bilinear(c):
                r0 = 80 * c
                # chunks 0-1: route replication DMAs via the Pool queue, which
                # is nearly empty during phase 1 — they fire as soon as h0/h1
                # m-tile 0/1 land, well before the PE finishes the
                # projections. SP/ACT are still draining phase-1 triggers.
                early = c == 0
                q_wb = nc.gpsimd if early else nc.sync
                q_hd = nc.gpsimd if early else nc.sync
                q_hr = nc.gpsimd if early else nc.scalar
                wbt = (head_pool if c == 0 else wb_pool).tile(
                    [128, KT_BIL, CS], BF16, tag="wbt"
                )
                q_wb.dma_start(out=wbt, in_=wbp[c])
                # h1dup: hd[p, j, b] = h1[80c + 16j + p%16, b] (DRAM src:
                # dim0 stride-0 replication only legal on DRAM-side APs)
                hd = (head_pool if c == 0 else dup_pool).tile(
                    [128, 5, BL], BF16, tag="hd"
                )
                for j in range(5):
                    row = r0 + 16 * j
                    p0, mt0 = row % 128, row // 128
                    src = (
                        h1_dram[p0 : p0 + 16, mt0, :]
                        .unsqueeze(0)
                        .broadcast_to([8, 16, BL])
                    )
                    q_hd.dma_start(out=hd[:, j, :], in_=src)
                # h0rep halves: hr[p, ii, b] = h0[80c + 8*(i0+ii) + p//16, b]
                hr_tiles = []
                for half in range(2):
                    if c == 0 and half == 0:
                        pool = head_pool
                    else:
                        pool = rep0_pool if half == 0 else rep1_pool
                    hr = pool.tile([128, 5, BL], BF16, tag=f"hr{half}")
                    for ii in range(5):
                        row = r0 + 8 * (5 * half + ii)
                        p0, kt0 = row % 128, row // 128
                        src = (
                            h0s[p0 : p0 + 8, kt0, :]
                            .unsqueeze(1)
                            .broadcast_to([8, 16, BL])
                        )
                        q_hr.dma_start(out=hr[:, ii, :], in_=src)
                    hr_tiles.append(hr)
                zps = zpsum_pool.tile([CS, BL], F32, tag="zps")
                for i in range(10):
                    hr = hr_tiles[i // 5]
                    nd = ND_PAT[i]
                    for h in range(NH):
                        cols = slice(h * 512, (h + 1) * 512)
                        pt = p_pool.tile([128, 5, 512], BF16, tag="pt")
                        h0b = hr[:, i % 5, cols].unsqueeze(1)
                        nc.vector.tensor_mul(
                            pt[:, :nd, :],
                            h0b.broadcast_to([128, nd, 512]),
                            hd[:, :nd, cols],
                        )
                        nc.gpsimd.tensor_mul(
                            pt[:, nd:, :],
                            h0b.broadcast_to([128, 5 - nd, 512]),
                            hd[:, nd:, cols],
                        )
                        for j in range(5):
                            kt = 5 * i + j
                            nc.tensor.matmul(
                                out=zps[:, cols],
                                lhsT=wbt[:, kt, :],
                                rhs=pt[:, j, :],
                                start=(kt == 0),
                                stop=(kt == KT_BIL - 1),
                            )
                return zps

            def emit_post(c, zps):
                # a = |z+bb|, g = sign(z+bb), s = sqrt(a), zst = s*g
                av = post_pool.tile([CS, BL], BF16, tag="av")
                nc.scalar.activation(
                    out=av, in_=zps, func=AF.Abs, bias=bbs[:, c : c + 1], scale=1.0
                )
                # nsq[b] = sum_o |z+bb| ( = ||signsqrt(z)||^2 ), then the
                # norm chain, emitted before gv/sv so the in-order ACT queue
                # runs av -> nrm first (the norm chain is the critical path
                # of the last chunk's tail)
                nps = nsq_psum.tile([1, BL], F32, tag="nps")
                for h in range(NH):
                    nc.tensor.matmul(
                        out=nps[:, h * 512 : (h + 1) * 512],
                        lhsT=ones80,
                        rhs=av[:, h * 512 : (h + 1) * 512],
                        start=True,
                        stop=True,
                    )
                nrm = nrm_pool.tile([1, BL], BF16, tag="nrm")
                nc.scalar.activation(out=nrm, in_=nps, func=AF.Sqrt, bias=eps_sq)
                rn = nrm_pool.tile([1, BL], BF16, tag="rn")
                with nc.allow_low_precision(reason="rn scales bf16 z anyway"):
                    nc.vector.reciprocal(rn, nrm)
                nc.sync.dma_start(out=rn_dram[c : c + 1, :], in_=rn)
                rnb = rnb_pool.tile([CS, BL], BF16, tag="rnb")
                nc.sync.dma_start(
                    out=rnb, in_=rn_dram[c : c + 1, :].partition_broadcast(CS)
                )
                gv = post_pool.tile([CS, BL], BF16, tag="gv")
                nc.scalar.activation(
                    out=gv, in_=zps, func=AF.Sign, bias=bbs[:, c : c + 1], scale=1.0
                )
                sv = post_pool.tile([CS, BL], BF16, tag="sv")
                nc.scalar.activation(out=sv, in_=av, func=AF.Sqrt)
                zst = zs_pool.tile([CS, BL], BF16, tag="zst")
                nc.gpsimd.tensor_mul(zst, sv, gv)
                nc.gpsimd.tensor_mul(zst, zst, rnb)
                for off, p0, kt, n in _h_row_segments(80 * c, CS):
                    dst = zbig_slice(kt)
                    nc.scalar.dma_start(
                        out=dst[p0 : p0 + n, :], in_=zst[off : off + n, :]
                    )

            wo_pool = p23.enter_context(tc.tile_pool(name="wo", bufs=3))
            o_pool = p23.enter_context(tc.tile_pool(name="opool", bufs=2))

            def emit_out_tile(mt, wot, ps_open=None):
                """Phase-4 m-tile. If ps_open is given, its two PSUM halves
                already hold the kt 0..7 partial accumulation."""
                m0 = mt * 128
                mw = min(128, D_OUT - m0)
                ot = o_pool.tile([128, BL], F32, tag="ot")
                for h in range(NH):
                    if ps_open is None:
                        ps = mm_psum.tile([128, 512], F32, tag="mmps")
                        kt_start = 0
                    else:
                        ps = ps_open[h]
                        kt_start = 8
                    for kt in range(kt_start, MT_H):
                        nc.tensor.matmul(
                            out=ps,
                            lhsT=wot[:, kt, :],
                            rhs=zbig_slice(kt, slice(h * 512, (h + 1) * 512)),
                            start=(kt == 0),
                            stop=(kt == MT_H - 1),
                        )
                    nc.scalar.activation(
                        out=ot[:, h * 512 : (h + 1) * 512],
                        in_=ps,
                        func=AF.Identity,
                        bias=bouts[:, mt : mt + 1],
                        scale=1.0,
                    )
                    if mt >= MT_O - 2:
                        # stream the tail stores per-half
                        nc.sync.dma_start(
                            out=outT[m0 : m0 + mw, h * 512 : (h + 1) * 512],
                            in_=ot[:mw, h * 512 : (h + 1) * 512],
                        )
                if mt < MT_O - 2:
                    nc.sync.dma_start(out=outT[m0 : m0 + mw, :], in_=ot[:mw, :])

            prev = emit_bilinear(0)
            for c in range(1, CHUNKS):
                cur = emit_bilinear(c)
                emit_post(c - 1, prev)
                prev = cur
            # Phase-4 head start: accumulate m-tile 0 over kt 0..7 (rows
            # 0..1023 = chunks 0..12, long since final) NOW, so the PE has
            # work while the last chunk's norm chain drains.
            wot0 = wo_pool.tile([128, MT_H, 128], BF16, tag="wot")
            nc.sync.dma_start(out=wot0, in_=woutp[0])
            ps_open = []
            for h in range(NH):
                ps = mm_psum.tile([128, 512], F32, tag="mmps")
                for kt in range(8):
                    nc.tensor.matmul(
                        out=ps,
                        lhsT=wot0[:, kt, :],
                        rhs=zbig_slice(kt, slice(h * 512, (h + 1) * 512)),
                        start=(kt == 0),
                        stop=False,
                    )
                ps_open.append(ps)
            emit_post(CHUNKS - 1, prev)

            # ---- Phase 4: out^T = Wout^T-style matmul + bout ----
            emit_out_tile(0, wot0, ps_open=ps_open)
            for mt in range(1, MT_O):
                wot = wo_pool.tile([128, MT_H, 128], BF16, tag="wot")
                nc.sync.dma_start(out=wot, in_=woutp[mt])
                emit_out_tile(mt, wot)


_PROGRAM = None


def _get_program():
    global _PROGRAM
    if _PROGRAM is None:
        _PROGRAM = build_program()
    return _PROGRAM


def prep_weights(W0, b0, W1, b1, Wb, bb, Wout, bout):
    bf = ml_dtypes.bfloat16
    # w0p[mt, p, kt, m] = W0pad[kt*128 + p, mt*128 + m]
    W0p = np.zeros((D_IN, MM_PAD), np.float32)
    W0p[:, :MM] = W0
    w0p = np.ascontiguousarray(
        W0p.reshape(K_IN, 128, MT_H, 128).transpose(2, 1, 0, 3), dtype=bf
    )
    W1p = np.zeros((D_IN, MM_PAD), np.float32)
    W1p[:, :MM] = W1
    w1p = np.ascontiguousarray(
        W1p.reshape(K_IN, 128, MT_H, 128).transpose(2, 1, 0, 3), dtype=bf
    )
    # wbp[c, p, kt, o] = Wb[c, o, 8*(kt//5) + p//16, 16*(kt%5) + p%16]
    p = np.arange(128)
    kt = np.arange(KT_BIL)
    s_idx = 8 * (kt[:, None] // 5) + p[None, :] // 16  # [50, 128]
    t_idx = 16 * (kt[:, None] % 5) + p[None, :] % 16
    wbp = np.asarray(Wb)[:, :, s_idx, t_idx]  # [C, o, 50, 128]
    wbp = np.ascontiguousarray(wbp.transpose(0, 3, 2, 1), dtype=bf)
    # woutp[mt, p, kt, m] = Woutpad[kt*128 + p, mt*128 + m]
    Woutp = np.zeros((MM_PAD, D_OUT_PAD), np.float32)
    Woutp[:MM, :D_OUT] = Wout
    woutp = np.ascontiguousarray(
        Woutp.reshape(MT_H, 128, MT_O, 128).transpose(2, 1, 0, 3), dtype=bf
    )
    b0p = np.zeros(MM_PAD, np.float32)
    b0p[:MM] = b0
    b0c = np.ascontiguousarray(b0p.reshape(MT_H, 128).T)
    b1p = np.zeros(MM_PAD, np.float32)
    b1p[:MM] = b1
    b1c = np.ascontiguousarray(b1p.reshape(MT_H, 128).T)
    bbT = np.ascontiguousarray(np.asarray(bb, np.float32).T)
    boutp = np.zeros(D_OUT_PAD, np.float32)
    boutp[:D_OUT] = bout
    boutc = np.ascontiguousarray(boutp.reshape(MT_O, 128).T)
    return dict(
        w0p=w0p, w1p=w1p, wbp=wbp, woutp=woutp, b0c=b0c, b1c=b1c, bbT=bbT,
        boutc=boutc,
    )


def make_in_maps(x0, x1, weights):
    bf = ml_dtypes.bfloat16
    x0T = np.ascontiguousarray(np.asarray(x0).T, dtype=bf)
    x1T = np.ascontiguousarray(np.asarray(x1).T, dtype=bf)
    in_maps = []
    for r in range(N_CORES):
        sl = slice(r * BL, (r + 1) * BL)
        m = dict(weights)
        m["x0T"] = np.ascontiguousarray(x0T[:, sl])
        m["x1T"] = np.ascontiguousarray(x1T[:, sl])
        in_maps.append(m)
    return in_maps


def run(x0, x1, weights, **kwargs):
    nc = _get_program()
    in_maps = make_in_maps(x0, x1, weights)
    res = run_bass_kernel_spmd(nc, in_maps, core_ids=list(range(N_CORES)), **kwargs)
    out = np.empty((B, D_OUT), np.float32)
    for r in range(N_CORES):
        out[r * BL : (r + 1) * BL, :] = res.results[r]["outT"].T
    return out, res


def kernel(x0, x1, W0, b0, W1, b1, Wb, bb, Wout, bout):
    weights = prep_weights(W0, b0, W1, b1, Wb, bb, Wout, bout)
    out, _ = run(x0, x1, weights)
    return out


# ---- timed runner (no NTFF hook in this container: wall-clock the PJRT
# executable with device-resident inputs, minus dispatch overhead) ----

def _make_sharded_callable(nc, in_maps):
    import jax
    import numpy as _np
    from jax.sharding import Mesh, PartitionSpec, NamedSharding
    from jax.experimental.shard_map import shard_map
    from concourse import bass2jax as b2j
    from concourse import mybir as _mybir

    b2j.install_neuronx_cc_hook()
    n_cores = len(in_maps)
    partition_name = nc.partition_id_tensor.name if nc.partition_id_tensor else None
    in_names, out_names, out_avals, zero_outs = [], [], [], []
    for alloc in nc.m.functions[0].allocations:
        if not isinstance(alloc, _mybir.MemoryLocationSet):
            continue
        name = alloc.memorylocations[0].name
        if alloc.kind == "ExternalInput":
            if name != partition_name:
                in_names.append(name)
        elif alloc.kind == "ExternalOutput":
            shape = tuple(alloc.tensor_shape)
            dtype = _mybir.dt.np(alloc.dtype)
            out_names.append(name)
            out_avals.append(jax.core.ShapedArray(shape, dtype))
            zero_outs.append(_np.zeros(shape, dtype))
    n_params = len(in_names)
    in_names_all = list(in_names) + list(out_names)
    if partition_name is not None:
        in_names_all.append(partition_name)

    def _body(*args):
        operands = list(args)
        if partition_name is not None:
            operands.append(b2j.partition_id_tensor())
        outs = b2j._bass_exec_p.bind(
            *operands,
            out_avals=tuple(out_avals),
            in_names=tuple(in_names_all),
            out_names=tuple(out_names),
            lowering_input_output_aliases=(),
            sim_require_finite=True,
            sim_require_nnan=True,
            nc=nc,
        )
        return tuple(outs)

    devices = jax.devices()[:n_cores]
    mesh = Mesh(_np.asarray(devices), ("core",))
    spec = PartitionSpec("core")
    in_specs = (spec,) * (n_params + len(out_names))
    out_specs = (spec,) * len(out_names)
    n_outs = len(out_names)
    donate = tuple(range(n_params, n_params + n_outs))
    sharded = jax.jit(
        shard_map(_body, mesh=mesh, in_specs=in_specs, out_specs=out_specs,
                  check_rep=False),
        keep_unused=True,
        donate_argnums=donate,
    )
    sh = NamedSharding(mesh, spec)
    concat_in = [
        jax.device_put(
            _np.concatenate([_np.asarray(in_maps[c][n]) for c in range(n_cores)], 0), sh
        )
        for n in in_names
    ]
    state = {"outs": None}

    def _fresh_zeros():
        return [
            jax.device_put(_np.zeros((n_cores * z.shape[0], *z.shape[1:]), z.dtype), sh)
            for z in zero_outs
        ]

    def call():
        # outputs are donated back in as the next call's output seeds; the
        # kernel fully overwrites every output, so contents don't matter
        seeds = state["outs"] if state["outs"] is not None else _fresh_zeros()
        outs = sharded(*concat_in, *seeds)
        state["outs"] = list(outs)
        return outs
    return call, out_names, out_avals


def bench(x0, x1, weights, iters=30):
    """Returns (out, per_iter_seconds_list)."""
    import jax, time
    nc = _get_program()
    in_maps = make_in_maps(x0, x1, weights)
    call, out_names, out_avals = _make_sharded_callable(nc, in_maps)
    res = call()
    jax.block_until_ready(res)
    times = []
    for _ in range(iters):
        t0 = time.perf_counter_ns()
        r = call()
        jax.block_until_ready(r)
        times.append((time.perf_counter_ns() - t0))
    out_arr = np.asarray(res[out_names.index("outT")]).reshape(N_CORES, D_OUT, BL)
    out = np.empty((B, D_OUT), np.float32)
    for r_ in range(N_CORES):
        out[r_ * BL : (r_ + 1) * BL, :] = out_arr[r_].T
    return out, times


def bench_overhead(iters=30):
    """Dispatch overhead baseline: trivial 1-DMA kernel through same path."""
    import jax, time
    global _TINY
    try:
        nc = _TINY
    except NameError:
        nc = None
    if nc is None:
        nc = bacc.Bacc("TRN2", target_bir_lowering=False, debug=False)
        a = nc.dram_tensor("a", [128, 16], F32, kind="ExternalInput").ap()
        o = nc.dram_tensor("o", [128, 16], F32, kind="ExternalOutput").ap()
        with tile.TileContext(nc) as tc:
            with tc.tile_pool(name="p", bufs=1) as pool:
                t = pool.tile([128, 16], F32)
                nc.sync.dma_start(out=t, in_=a)
                nc.sync.dma_start(out=o, in_=t)
        nc.compile()
        _TINY = nc
    in_maps = [dict(a=np.zeros((128, 16), np.float32)) for _ in range(N_CORES)]
    call, _, _ = _make_sharded_callable(nc, in_maps)
    res = call()
    jax.block_until_ready(res)
    times = []
    for _ in range(iters):
        t0 = time.perf_counter_ns()
        r = call()
        jax.block_until_ready(r)
        times.append(time.perf_counter_ns() - t0)
    return times


def bench_async(x0, x1, weights, iters=50):
    """Amortized per-iter time: N async dispatches, single block at the end."""
    import jax, time
    nc = _get_program()
    in_maps = make_in_maps(x0, x1, weights)
    call, out_names, out_avals = _make_sharded_callable(nc, in_maps)
    res = call()
    jax.block_until_ready(res)
    # pipeline warmup
    rs = [call() for _ in range(5)]
    jax.block_until_ready(rs)
    t0 = time.perf_counter_ns()
    rs = [call() for _ in range(iters)]
    jax.block_until_ready(rs)
    dt = time.perf_counter_ns() - t0
    out_arr = np.asarray(res[out_names.index("outT")]).reshape(N_CORES, D_OUT, BL)
    out = np.empty((B, D_OUT), np.float32)
    for r_ in range(N_CORES):
        out[r_ * BL : (r_ + 1) * BL, :] = out_arr[r_].T
    return out, dt / iters


def bench_async_overhead(iters=50):
    import jax, time
    global _TINY2
    try:
        nc = _TINY2
    except NameError:
        nc = None
    if nc is None:
        nc = bacc.Bacc("TRN2", target_bir_lowering=False, debug=False)
        a = nc.dram_tensor("a", [128, 16], F32, kind="ExternalInput").ap()
        o = nc.dram_tensor("o", [128, 16], F32, kind="ExternalOutput").ap()
        with tile.TileContext(nc) as tc:
            with tc.tile_pool(name="p", bufs=1) as pool:
                t = pool.tile([128, 16], F32)
                nc.sync.dma_start(out=t, in_=a)
                nc.sync.dma_start(out=o, in_=t)
        nc.compile()
        _TINY2 = nc
    in_maps = [dict(a=np.zeros((128, 16), np.float32)) for _ in range(N_CORES)]
    call, _, _ = _make_sharded_callable(nc, in_maps)
    import jax as _j
    _j.block_until_ready(call())
    rs = [call() for _ in range(5)]
    _j.block_until_ready(rs)
    import time as _t
    t0 = _t.perf_counter_ns()
    rs = [call() for _ in range(iters)]
    _j.block_until_ready(rs)
    return (_t.perf_counter_ns() - t0) / iters


def _make_tiny_callable():
    global _TINY3
    try:
        nc = _TINY3
    except NameError:
        nc = None
    if nc is None:
        nc = bacc.Bacc("TRN2", target_bir_lowering=False, debug=False)
        a = nc.dram_tensor("a", [128, 16], F32, kind="ExternalInput").ap()
        o = nc.dram_tensor("o", [128, 16], F32, kind="ExternalOutput").ap()
        with tile.TileContext(nc) as tc:
            with tc.tile_pool(name="p", bufs=1) as pool:
                t = pool.tile([128, 16], F32)
                nc.sync.dma_start(out=t, in_=a)
                nc.sync.dma_start(out=o, in_=t)
        nc.compile()
        _TINY3 = nc
    in_maps = [dict(a=np.zeros((128, 16), np.float32)) for _ in range(N_CORES)]
    call, _, _ = _make_sharded_callable(nc, in_maps)
    return call

